# revision 1
# baseline (speedup 1.0000x reference)
"""CFConv (continuous-filter convolution) distributed Bass kernel for 8 trn2 cores.

    y = segment_sum(x[idx_j] * Wij, idx_i, N)    x:[N,F] Wij:[E,F] idx:[E]

Strategy (idx_i is sorted):
  - Atoms are grouped into blocks of 128; blocks are split contiguously across
    the 8 cores (49 blocks/core).  Each core owns the edges whose idx_i falls in
    its blocks, so per-core outputs are disjoint -> no collectives.
  - Per core: stream edge tiles of 128.  x rows are fetched with the optimized
    GPSIMD dma_gather (int16 indices; the atom table is split in two halves so
    indices fit in int16).  Edge features are multiplied by the (host-permuted,
    fp16) filter stream on DVE, then segment-summed into a 128-atom block via a
    one-hot selection matmul on the tensor engine with PSUM accumulation
    (S[e,a] = (iota[a] == idx_i[e] - block_base), built with one DVE
    tensor_scalar is_equal per tile).
  - Host does index bookkeeping only: it permutes/pads Wij + index streams into
    the tile order the kernel consumes, and slices the outputs back together.

The SPMD program is identical on all cores; tile counts are padded to the
cross-core max (zero-filter edges contribute nothing).
"""

import math

import numpy as np

N_CORES = 8
P = 128
HALF_SPLIT = 25088  # atom-table split so gather indices fit in int16
DEBUG_DISABLE = set()  # {"gather", "compute", "mul", "final_dma"} for bisection


# ---------------------------------------------------------------- host planning


def _plan_and_pack(x, Wij, idx_i, idx_j):
    """Compute the shared (cross-core uniform) tile schedule and pack per-core
    input streams."""
    N, F = x.shape
    E = Wij.shape[0]
    nb_global = math.ceil(N / P)  # atom blocks overall
    nbc = math.ceil(nb_global / N_CORES)  # blocks per core
    nb_pad = nbc * N_CORES

    # edge ranges per global block (idx_i sorted)
    bounds = np.searchsorted(idx_i, np.arange(nb_pad + 1) * P)
    lo_mask = idx_j < HALF_SPLIT

    # counts[c, j, h]
    counts = np.zeros((N_CORES, nbc, 2), dtype=np.int64)
    for b in range(nb_pad):
        c, j = divmod(b, nbc)
        s, e = bounds[b], bounds[b + 1]
        nlo = int(lo_mask[s:e].sum())
        counts[c, j, 0] = nlo
        counts[c, j, 1] = (e - s) - nlo

    # uniform tiles per (local block, half) = cross-core max
    T = np.ceil(counts.max(axis=0) / P).astype(np.int64)  # [nbc, 2]

    # group blocks into windows of ~WT_TARGET tiles
    WT_TARGET = 96
    windows = []  # list of lists of local block ids
    cur, cur_t = [], 0
    for j in range(nbc):
        tj = int(T[j, 0] + T[j, 1])
        if cur and cur_t + tj > WT_TARGET:
            windows.append(cur)
            cur, cur_t = [], 0
        cur.append(j)
        cur_t += tj
    if cur:
        windows.append(cur)

    # per-window slot layout: [all lo tiles (block-major)] ++ [all hi tiles]
    sched = []
    woff = 0  # global slot offset
    lo_off_g = 0  # global lo-slot offset (for idx stream columns)
    hi_off_g = 0
    for wblocks in windows:
        w_lo = int(sum(T[j, 0] for j in wblocks))
        w_hi = int(sum(T[j, 1] for j in wblocks))
        blocks = []
        lo_cursor, hi_cursor = 0, 0
        for j in wblocks:
            blocks.append(
                dict(
                    j=j,
                    t_lo=int(T[j, 0]),
                    t_hi=int(T[j, 1]),
                    lo_slot=lo_cursor,  # window-local slot of first lo tile
                    hi_slot=w_lo + hi_cursor,
                )
            )
            lo_cursor += int(T[j, 0])
            hi_cursor += int(T[j, 1])
        sched.append(
            dict(
                blocks=blocks,
                w_lo=w_lo,
                w_hi=w_hi,
                wt=w_lo + w_hi,
                woff=woff,
                lo_off=lo_off_g,
                hi_off=hi_off_g,
            )
        )
        woff += w_lo + w_hi
        lo_off_g += w_lo
        hi_off_g += w_hi

    t_tot = woff
    l_lo = lo_off_g
    l_hi = hi_off_g

    # ---- pack per-core streams
    per_core = []
    for c in range(N_CORES):
        wij_stream = np.zeros((t_tot * P, F), dtype=np.float16)
        rel_stream = np.zeros(t_tot * P, dtype=np.float32)
        loidx_stream = np.zeros(l_lo * P, dtype=np.int16)
        hiidx_stream = np.zeros(l_hi * P, dtype=np.int16)
        for w in sched:
            for blk in w["blocks"]:
                j = blk["j"]
                b = c * nbc + j
                s, e = bounds[b], bounds[b + 1]
                mask = lo_mask[s:e]
                for h in (0, 1):
                    el = np.arange(s, e)[mask if h == 0 else ~mask]
                    m = len(el)
                    if m == 0:
                        continue
                    gslot = w["woff"] + (blk["lo_slot"] if h == 0 else blk["hi_slot"])
                    pos = gslot * P
                    wij_stream[pos : pos + m] = Wij[el]
                    rel_stream[pos : pos + m] = (idx_i[el] - b * P).astype(np.float32)
                    if h == 0:
                        hpos = (w["lo_off"] + blk["lo_slot"]) * P
                        loidx_stream[hpos : hpos + m] = idx_j[el].astype(np.int16)
                    else:
                        hpos = (w["hi_off"] + blk["hi_slot"] - w["w_lo"]) * P
                        hiidx_stream[hpos : hpos + m] = (idx_j[el] - HALF_SPLIT).astype(
                            np.int16
                        )

        wij_t = (
            wij_stream.reshape(t_tot, P, F).transpose(1, 0, 2).reshape(P, t_tot * F)
        )
        rel_t = rel_stream.reshape(t_tot, P).T.copy()
        idx_lo = np.tile(loidx_stream.reshape(-1, 16).T, (8, 1)) if l_lo else None
        idx_hi = np.tile(hiidx_stream.reshape(-1, 16).T, (8, 1)) if l_hi else None
        per_core.append(
            dict(wij_t=wij_t, rel_t=rel_t, idx_lo=idx_lo, idx_hi=idx_hi)
        )

    meta = dict(
        N=N,
        F=F,
        E=E,
        nbc=nbc,
        t_tot=t_tot,
        l_lo=l_lo,
        l_hi=l_hi,
        sched=sched,
    )
    return meta, per_core


# ---------------------------------------------------------------- device kernel


def _build(meta, reps=1):
    import concourse.bacc as bacc
    import concourse.mybir as mybir
    import concourse.tile as tile

    F = meta["F"]
    N = meta["N"]
    nbc = meta["nbc"]
    t_tot = meta["t_tot"]
    l_lo = meta["l_lo"]
    l_hi = meta["l_hi"]
    sched = meta["sched"]
    n_hi = N - HALF_SPLIT

    f32 = mybir.dt.float32
    f16 = mybir.dt.float16
    i16 = mybir.dt.int16

    max_wt = max(w["wt"] for w in sched)
    max_wlo = max(w["w_lo"] for w in sched)
    max_whi = max(w["w_hi"] for w in sched)

    nc = bacc.Bacc(None, target_bir_lowering=False, num_swdge_queues=4)
    x_lo = nc.declare_dram_parameter("x_lo", [N, 2 * F], f16, isOutput=False)
    x_hi = nc.declare_dram_parameter("x_hi", [n_hi, 2 * F], f16, isOutput=False)
    wij_t = nc.declare_dram_parameter("wij_t", [P, t_tot * F], f16, isOutput=False)
    rel_t = nc.declare_dram_parameter("rel_t", [P, t_tot], f32, isOutput=False)
    idx_lo = nc.declare_dram_parameter("idx_lo", [P, l_lo * 8], i16, isOutput=False)
    idx_hi = nc.declare_dram_parameter("idx_hi", [P, l_hi * 8], i16, isOutput=False)
    iota = nc.declare_dram_parameter("iota", [P, P], f16, isOutput=False)
    y = nc.declare_dram_parameter("y", [nbc * P, F], f32, isOutput=True)

    with tile.TileContext(nc) as tc:
        with (
            tc.tile_pool(name="persist", bufs=1) as persist,
            tc.tile_pool(name="io_idx", bufs=2) as pool_idx,
            tc.tile_pool(name="io_w", bufs=2) as pool_w,
            tc.tile_pool(name="io_xg", bufs=24) as pool_xg,
            tc.tile_pool(name="xw", bufs=24) as pool_xw,
            tc.tile_pool(name="sel", bufs=4) as pool_s,
            tc.tile_pool(name="psum", bufs=8, space="PSUM") as pool_psum,
        ):
            iota_sb = persist.tile([P, P], f16)
            nc.sync.dma_start(iota_sb[:], iota[:])
            y_sb = persist.tile([P, nbc * F], f32)
            nc.vector.memset(y_sb[:], 0.0)

            for _rep in range(reps):
              for w in sched:
                  wt, w_lo, w_hi = w["wt"], w["w_lo"], w["w_hi"]
                  woff = w["woff"]

                  wij_sb = pool_w.tile([P, max_wt * F], f16, tag="wij")
                  nc.sync.dma_start(
                      wij_sb[:, : wt * F],
                      wij_t[:, woff * F : (woff + wt) * F],
                  )
                  rel_sb = pool_w.tile([P, max_wt], f32, tag="rel")
                  nc.sync.dma_start(rel_sb[:, :wt], rel_t[:, woff : woff + wt])

                  GCHUNK = 8  # tiles per dma_gather call / per mul
                  n_lo_ch = (w_lo + GCHUNK - 1) // GCHUNK
                  n_hi_ch = (w_hi + GCHUNK - 1) // GCHUNK
                  ilo_sb = ihi_sb = None
                  if w_lo and "gather" not in DEBUG_DISABLE:
                      ilo_sb = pool_idx.tile([P, max_wlo * 8], i16, tag="ilo")
                      nc.sync.dma_start(
                          ilo_sb[:, : w_lo * 8],
                          idx_lo[:, w["lo_off"] * 8 : (w["lo_off"] + w_lo) * 8],
                      )
                  if w_hi and "gather" not in DEBUG_DISABLE:
                      ihi_sb = pool_idx.tile([P, max_whi * 8], i16, tag="ihi")
                      nc.sync.dma_start(
                          ihi_sb[:, : w_hi * 8],
                          idx_hi[:, w["hi_off"] * 8 : (w["hi_off"] + w_hi) * 8],
                      )
                  # chunk ch covers window slots [ch*GCHUNK, ...) of its half
                  xw_tiles = []  # (slot0, width, xw_tile)
                  qn = 0
                  for ch in range(n_lo_ch + n_hi_ch):
                      if ch < n_lo_ch:
                          g0 = ch * GCHUNK
                          cw = min(GCHUNK, w_lo - g0)
                          src, isb, s0 = x_lo, ilo_sb, g0
                      else:
                          g0 = (ch - n_lo_ch) * GCHUNK
                          cw = min(GCHUNK, w_hi - g0)
                          src, isb, s0 = x_hi, ihi_sb, w_lo + g0
                      xg_c = pool_xg.tile([P, GCHUNK, 2 * F], f16, tag="xg")
                      if "gather" in DEBUG_DISABLE:
                          nc.vector.memset(
                              xg_c[:, :cw, :].rearrange("p t f -> p (t f)"), 0.0
                          )
                      else:
                          nc.gpsimd.dma_gather(
                              xg_c[:, :cw, :],
                              src[:],
                              isb[:, g0 * 8 : (g0 + cw) * 8],
                              cw * P,
                              cw * P,
                              2 * F,
                              queue_num=qn,
                          )
                          qn = (qn + 1) % 4
                      xw_c = pool_xw.tile([P, GCHUNK * F], f16, tag="xw")
                      if "mul" in DEBUG_DISABLE:
                          nc.vector.memset(xw_c[:, : cw * F], 0.0)
                      else:
                          nc.vector.tensor_tensor(
                              out=xw_c[:, : cw * F].rearrange(
                                  "p (t f) -> p t f", f=F
                              ),
                              in0=xg_c[:, :cw, :F],
                              in1=wij_sb[:, s0 * F : (s0 + cw) * F].rearrange(
                                  "p (t f) -> p t f", f=F
                              ),
                              op=mybir.AluOpType.mult,
                          )
                      xw_tiles.append((s0, cw, xw_c))

                  def xw_slice(s):
                      for s0, cw, t in xw_tiles:
                          if s0 <= s < s0 + cw:
                              return t[:, (s - s0) * F : (s - s0 + 1) * F]
                      raise AssertionError(s)

                  for blk in (w["blocks"] if "compute" not in DEBUG_DISABLE else []):
                      ntiles = blk["t_lo"] + blk["t_hi"]
                      if ntiles == 0:
                          continue
                      slots = [blk["lo_slot"] + t for t in range(blk["t_lo"])] + [
                          blk["hi_slot"] + t for t in range(blk["t_hi"])
                      ]
                      ps = pool_psum.tile([P, F], f32, tag="ps")
                      for k, s in enumerate(slots):
                          sel = pool_s.tile([P, P], f16, tag="sel")
                          nc.vector.tensor_scalar(
                              out=sel[:],
                              in0=iota_sb[:],
                              scalar1=rel_sb[:, s : s + 1],
                              scalar2=None,
                              op0=mybir.AluOpType.is_equal,
                          )
                          nc.tensor.matmul(
                              out=ps[:],
                              lhsT=sel[:],
                              rhs=xw_slice(s),
                              start=(k == 0),
                              stop=(k == ntiles - 1),
                          )
                      j = blk["j"]
                      nc.scalar.copy(out=y_sb[:, j * F : (j + 1) * F], in_=ps[:])

            if "final_dma" not in DEBUG_DISABLE:
                nc.sync.dma_start(
                    y[:].rearrange("(j p) f -> p j f", p=P),
                    y_sb[:].rearrange("p (j f) -> p j f", f=F),
                )
            else:
                nc.sync.dma_start(y[:P, :], y_sb[:, :F])
    nc.compile()
    return nc


# ---------------------------------------------------------------- entry point


def prepare(x, Wij, idx_i, idx_j):
    """Host planning + program build.  Returns (nc, in_maps, meta)."""
    x = np.ascontiguousarray(np.asarray(x, dtype=np.float32))
    Wij = np.asarray(Wij, dtype=np.float32)
    idx_i = np.asarray(idx_i, dtype=np.int64)
    idx_j = np.asarray(idx_j, dtype=np.int64)

    meta, per_core = _plan_and_pack(x, Wij, idx_i, idx_j)
    nc = _build(meta)

    F = meta["F"]
    iota_np = np.broadcast_to(
        np.arange(P, dtype=np.float16), (P, P)
    ).copy()
    x16 = np.zeros((x.shape[0], 2 * F), dtype=np.float16)
    x16[:, :F] = x
    x_hi_np = np.ascontiguousarray(x16[HALF_SPLIT:])
    in_maps = []
    for c in range(N_CORES):
        pc = per_core[c]
        in_maps.append(
            {
                "x_lo": x16,
                "x_hi": x_hi_np,
                "wij_t": pc["wij_t"],
                "rel_t": pc["rel_t"],
                "idx_lo": pc["idx_lo"],
                "idx_hi": pc["idx_hi"],
                "iota": iota_np,
            }
        )
    return nc, in_maps, meta


def kernel(x, Wij, idx_i, idx_j):
    from concourse.bass_utils import run_bass_kernel_spmd

    nc, in_maps, meta = prepare(x, Wij, idx_i, idx_j)
    res = run_bass_kernel_spmd(nc, in_maps, core_ids=list(range(N_CORES)))
    N = meta["N"]
    y = np.concatenate([res.results[c]["y"] for c in range(N_CORES)], axis=0)
    return np.ascontiguousarray(y[:N])



# revision 6
# speedup vs baseline: 1.2319x; 1.2319x over previous
"""CFConv (continuous-filter convolution) distributed Bass kernel for 8 trn2 cores.

    y = segment_sum(x[idx_j] * Wij, idx_i, N)    x:[N,F] Wij:[E,F] idx:[E]

Strategy (idx_i is sorted):
  - Atoms are grouped into blocks of 128; blocks are split contiguously across
    the 8 cores (49 blocks/core).  Each core owns the edges whose idx_i falls in
    its blocks, so per-core outputs are disjoint -> no collectives.
  - Both x and Wij are int8-quantized with per-row scales (validated rel err
    ~6e-3 vs the 2e-2 budget).  The combined per-edge scale
    s_e = s_w[e] * s_x[idx_j[e]] is folded into the segment-sum selection
    matrix, so no dequantization instructions are needed: the DVE multiplies
    the gathered int8 x tile by the int8 Wij tile directly into f16.
  - x ships as a QUAD atom table ([N/4, 4F] i8: row t holds atoms 4t..4t+3) so
    the 256B GPSIMD dma_gather granule carries no padding and the int16 gather
    index (j>>2 < 12500) fits easily.  Edges are partitioned into four streams
    per atom block by j&3; a stream's tiles multiply the matching quarter of
    the gathered rows, so no per-edge select is needed.
  - Per core, per edge tile of 128: gather x rows (GPSIMD dma_gather), DVE
    multiply x-quarter * w8 -> xw f16, build sel[e,a] = (iota[a]==rel_e)*s_e
    (one 2-op DVE tensor_scalar), and segment-sum via PSUM-accumulated
    matmuls on the tensor engine.  Output is written f16.
  - Gather indices ship compactly ([16, t_tot*8] i16) and are replicated to
    the 128-partition layout the gather wants by one broadcast DMA.
  - Host does index bookkeeping only: it permutes/quantizes the streams into
    tile order and slices the outputs back together.

The SPMD program is identical on all cores; tile counts are padded to the
cross-core max (zero-scale edges contribute nothing).
"""

import math

import numpy as np

N_CORES = 8
P = 128
NQ = 4  # atoms packed per gather row / per-block substreams
DEBUG_DISABLE = set()  # {"gather", "compute", "mul", "final_dma"} for bisection


# ---------------------------------------------------------------- host planning


def _quantize_rows(a):
    """Per-row symmetric int8 quantization. Returns (int8 values, f32 scales)."""
    absmax = np.abs(a).max(axis=1)
    scale = (absmax / 127.0).astype(np.float32)
    inv = np.where(absmax > 0, 127.0 / np.maximum(absmax, 1e-30), 0.0)
    q = np.clip(np.rint(a * inv[:, None]), -127, 127).astype(np.int8)
    return q, scale


def _plan_and_pack(x, Wij, idx_i, idx_j):
    """Compute the shared (cross-core uniform) tile schedule and pack per-core
    input streams."""
    N, F = x.shape
    E = Wij.shape[0]
    nb_global = math.ceil(N / P)  # atom blocks overall
    nbc = math.ceil(nb_global / N_CORES)  # blocks per core
    nb_pad = nbc * N_CORES

    w8, s_w = _quantize_rows(Wij)
    x8, s_x = _quantize_rows(x)

    # edge ranges per global block (idx_i sorted)
    bounds = np.searchsorted(idx_i, np.arange(nb_pad + 1) * P)
    q_of = (idx_j & (NQ - 1)).astype(np.int64)  # substream of each edge
    jq = (idx_j >> 2).astype(np.int16)  # quad-table gather index
    s_e = (s_w * s_x[idx_j]).astype(np.float32)  # combined scale

    # counts[c, j, q]
    counts = np.zeros((N_CORES, nbc, NQ), dtype=np.int64)
    for b in range(nb_pad):
        c, j = divmod(b, nbc)
        s, e = bounds[b], bounds[b + 1]
        if e > s:
            counts[c, j] = np.bincount(q_of[s:e], minlength=NQ)

    # uniform tiles per (local block, substream) = cross-core max
    T = np.ceil(counts.max(axis=0) / P).astype(np.int64)  # [nbc, NQ]

    # group blocks into windows of ~WT_TARGET tiles
    WT_TARGET = 96
    windows = []  # list of lists of local block ids
    cur, cur_t = [], 0
    for j in range(nbc):
        tj = int(T[j].sum())
        if cur and cur_t + tj > WT_TARGET:
            windows.append(cur)
            cur, cur_t = [], 0
        cur.append(j)
        cur_t += tj
    if cur:
        windows.append(cur)

    # per-window slot layout: [q0 tiles (block-major)] ++ [q1] ++ [q2] ++ [q3]
    sched = []
    woff = 0  # global slot offset
    for wblocks in windows:
        w_q = [int(sum(T[j, q] for j in wblocks)) for q in range(NQ)]
        qbase = np.concatenate([[0], np.cumsum(w_q)])  # window-local stream bases
        blocks = []
        cursor = [0] * NQ
        for j in wblocks:
            q_slot = [int(qbase[q] + cursor[q]) for q in range(NQ)]
            blocks.append(dict(j=j, t_q=[int(T[j, q]) for q in range(NQ)], q_slot=q_slot))
            for q in range(NQ):
                cursor[q] += int(T[j, q])
        sched.append(dict(blocks=blocks, w_q=w_q, wt=int(sum(w_q)), woff=woff))
        woff += int(sum(w_q))

    t_tot = woff

    # ---- pack per-core streams
    per_core = []
    for c in range(N_CORES):
        wij_stream = np.zeros((t_tot * P, F), dtype=np.int8)
        s_stream = np.zeros(t_tot * P, dtype=np.float32)
        rel_stream = np.zeros(t_tot * P, dtype=np.float32)
        idx_stream = np.zeros(t_tot * P, dtype=np.int16)
        for w in sched:
            for blk in w["blocks"]:
                j = blk["j"]
                b = c * nbc + j
                s, e = bounds[b], bounds[b + 1]
                if e == s:
                    continue
                el_all = np.arange(s, e)
                qa = q_of[s:e]
                for q in range(NQ):
                    el = el_all[qa == q]
                    m = len(el)
                    if m == 0:
                        continue
                    pos = (w["woff"] + blk["q_slot"][q]) * P
                    wij_stream[pos : pos + m] = w8[el]
                    s_stream[pos : pos + m] = s_e[el]
                    rel_stream[pos : pos + m] = (idx_i[el] - b * P).astype(np.float32)
                    idx_stream[pos : pos + m] = jq[el]

        wij_t = (
            wij_stream.reshape(t_tot, P, F).transpose(1, 0, 2).reshape(P, t_tot * F)
        )
        s_t = s_stream.reshape(t_tot, P).T.copy()
        rel_t = rel_stream.reshape(t_tot, P).T.copy()
        # compact 16-partition wrap (replicated to 128 on device)
        idx_t = idx_stream.reshape(-1, 16).T.copy()
        per_core.append(dict(wij_t=wij_t, s_t=s_t, rel_t=rel_t, idx_t=idx_t))

    meta = dict(N=N, F=F, E=E, nbc=nbc, t_tot=t_tot, sched=sched)
    return meta, per_core


# ---------------------------------------------------------------- device kernel


def _build(meta, reps=1):
    import concourse.bacc as bacc
    import concourse.mybir as mybir
    import concourse.tile as tile

    F = meta["F"]
    N = meta["N"]
    nbc = meta["nbc"]
    t_tot = meta["t_tot"]
    sched = meta["sched"]
    n_quad = (N + NQ - 1) // NQ

    f32 = mybir.dt.float32
    f16 = mybir.dt.float16
    i8 = mybir.dt.int8
    i16 = mybir.dt.int16

    max_wt = max(w["wt"] for w in sched)

    nc = bacc.Bacc(None, target_bir_lowering=False, num_swdge_queues=4)
    x_quad = nc.declare_dram_parameter("x_quad", [n_quad, NQ * F], i8, isOutput=False)
    wij_t = nc.declare_dram_parameter("wij_t", [P, t_tot * F], i8, isOutput=False)
    s_t = nc.declare_dram_parameter("s_t", [P, t_tot], f32, isOutput=False)
    rel_t = nc.declare_dram_parameter("rel_t", [P, t_tot], f32, isOutput=False)
    idx_t = nc.declare_dram_parameter("idx_t", [16, t_tot * 8], i16, isOutput=False)
    iota = nc.declare_dram_parameter("iota", [P, P], f16, isOutput=False)
    y = nc.declare_dram_parameter("y", [nbc * P, F], f16, isOutput=True)

    with tile.TileContext(nc) as tc:
        with (
            tc.tile_pool(name="persist", bufs=1) as persist,
            tc.tile_pool(name="io_w", bufs=2) as pool_w,
            tc.tile_pool(name="io_xg", bufs=24) as pool_xg,
            tc.tile_pool(name="xw", bufs=24) as pool_xw,
            tc.tile_pool(name="sel", bufs=4) as pool_s,
            tc.tile_pool(name="psum", bufs=8, space="PSUM") as pool_psum,
        ):
            iota_sb = persist.tile([P, P], f16)
            nc.sync.dma_start(iota_sb[:], iota[:])
            y_sb = persist.tile([P, nbc * F], f16)
            nc.vector.memset(y_sb[:], 0.0)

            # full idx stream resident in SBUF, replicated 16->128 partitions
            idx_sb = persist.tile([P, t_tot * 8], i16)
            if "gather" not in DEBUG_DISABLE:
                nc.sync.dma_start(
                    idx_sb[:].rearrange("(r p) x -> r p x", r=8),
                    idx_t[:].unsqueeze(0).broadcast_to([8, 16, t_tot * 8]),
                )

            for _rep in range(reps):
              for w in sched:
                  wt, w_q = w["wt"], w["w_q"]
                  woff = w["woff"]

                  wij_sb = pool_w.tile([P, max_wt * F], i8, tag="wij")
                  nc.sync.dma_start(
                      wij_sb[:, : wt * F],
                      wij_t[:, woff * F : (woff + wt) * F],
                  )
                  rel_sb = pool_w.tile([P, max_wt], f32, tag="rel")
                  nc.sync.dma_start(rel_sb[:, :wt], rel_t[:, woff : woff + wt])
                  s_sb = pool_w.tile([P, max_wt], f32, tag="s")
                  nc.sync.dma_start(s_sb[:, :wt], s_t[:, woff : woff + wt])

                  GCHUNK = 8  # tiles per dma_gather call / per mul
                  # chunks cover window slots without crossing substream bounds
                  chunks = []  # (slot0, width, quarter)
                  base = 0
                  for q in range(NQ):
                      for g0 in range(0, w_q[q], GCHUNK):
                          cw = min(GCHUNK, w_q[q] - g0)
                          chunks.append((base + g0, cw, q))
                      base += w_q[q]

                  xw_tiles = []  # (slot0, width, xw_tile)
                  qn = 0
                  for s0, cw, q in chunks:
                      xg_c = pool_xg.tile([P, GCHUNK, NQ * F], i8, tag="xg")
                      if "gather" in DEBUG_DISABLE:
                          nc.vector.memset(
                              xg_c[:, :cw, :].rearrange("p t f -> p (t f)"), 0
                          )
                      else:
                          nc.gpsimd.dma_gather(
                              xg_c[:, :cw, :],
                              x_quad[:],
                              idx_sb[:, (woff + s0) * 8 : (woff + s0 + cw) * 8],
                              cw * P,
                              cw * P,
                              NQ * F,
                              queue_num=qn,
                          )
                          qn = (qn + 1) % 4
                      xw_c = pool_xw.tile([P, GCHUNK * F], f16, tag="xw")
                      if "mul" in DEBUG_DISABLE:
                          nc.vector.memset(xw_c[:, : cw * F], 0.0)
                      else:
                          nc.vector.tensor_tensor(
                              out=xw_c[:, : cw * F].rearrange(
                                  "p (t f) -> p t f", f=F
                              ),
                              in0=xg_c[:, :cw, q * F : (q + 1) * F],
                              in1=wij_sb[:, s0 * F : (s0 + cw) * F].rearrange(
                                  "p (t f) -> p t f", f=F
                              ),
                              op=mybir.AluOpType.mult,
                          )
                      xw_tiles.append((s0, cw, xw_c))

                  def xw_slice(s):
                      for s0, cw, t in xw_tiles:
                          if s0 <= s < s0 + cw:
                              return t[:, (s - s0) * F : (s - s0 + 1) * F]
                      raise AssertionError(s)

                  for blk in (w["blocks"] if "compute" not in DEBUG_DISABLE else []):
                      ntiles = int(sum(blk["t_q"]))
                      if ntiles == 0:
                          continue
                      slots = []
                      for q in range(NQ):
                          slots += [blk["q_slot"][q] + t for t in range(blk["t_q"][q])]
                      ps = pool_psum.tile([P, F], f32, tag="ps")
                      for k, s in enumerate(slots):
                          sel = pool_s.tile([P, P], f16, tag="sel")
                          nc.vector.tensor_scalar(
                              out=sel[:],
                              in0=iota_sb[:],
                              scalar1=rel_sb[:, s : s + 1],
                              scalar2=s_sb[:, s : s + 1],
                              op0=mybir.AluOpType.is_equal,
                              op1=mybir.AluOpType.mult,
                          )
                          nc.tensor.matmul(
                              out=ps[:],
                              lhsT=sel[:],
                              rhs=xw_slice(s),
                              start=(k == 0),
                              stop=(k == ntiles - 1),
                          )
                      j = blk["j"]
                      nc.scalar.copy(out=y_sb[:, j * F : (j + 1) * F], in_=ps[:])

            if "final_dma" not in DEBUG_DISABLE:
                nc.sync.dma_start(
                    y[:].rearrange("(j p) f -> p j f", p=P),
                    y_sb[:].rearrange("p (j f) -> p j f", f=F),
                )
            else:
                nc.sync.dma_start(y[:P, :], y_sb[:, :F])
    nc.compile()
    return nc


# ---------------------------------------------------------------- entry point


def prepare(x, Wij, idx_i, idx_j):
    """Host planning + program build.  Returns (nc, in_maps, meta)."""
    x = np.ascontiguousarray(np.asarray(x, dtype=np.float32))
    Wij = np.asarray(Wij, dtype=np.float32)
    idx_i = np.asarray(idx_i, dtype=np.int64)
    idx_j = np.asarray(idx_j, dtype=np.int64)

    meta, per_core = _plan_and_pack(x, Wij, idx_i, idx_j)
    nc = _build(meta)

    N, F = meta["N"], meta["F"]
    n_quad = (N + NQ - 1) // NQ
    iota_np = np.broadcast_to(np.arange(P, dtype=np.float16), (P, P)).copy()
    x8, _ = _quantize_rows(x)
    x_quad_np = np.zeros((n_quad * NQ, F), dtype=np.int8)
    x_quad_np[:N] = x8
    x_quad_np = x_quad_np.reshape(n_quad, NQ * F)
    in_maps = []
    for c in range(N_CORES):
        pc = per_core[c]
        in_maps.append(
            {
                "x_quad": x_quad_np,
                "wij_t": pc["wij_t"],
                "s_t": pc["s_t"],
                "rel_t": pc["rel_t"],
                "idx_t": pc["idx_t"],
                "iota": iota_np,
            }
        )
    return nc, in_maps, meta


def kernel(x, Wij, idx_i, idx_j):
    from concourse.bass_utils import run_bass_kernel_spmd

    nc, in_maps, meta = prepare(x, Wij, idx_i, idx_j)
    res = run_bass_kernel_spmd(nc, in_maps, core_ids=list(range(N_CORES)))
    N = meta["N"]
    y = np.concatenate([res.results[c]["y"] for c in range(N_CORES)], axis=0)
    return np.ascontiguousarray(y[:N]).astype(np.float32)
